# revision 2
# baseline (speedup 1.0000x reference)
"""Additive attention (B=16, Q=128, K=1024, D=256, H=64) on 8 trn2 NeuronCores.

scores[b,q,k] = sum_h Wv[h] * tanh(qproj[b,q,h] + kproj[b,k,h]); softmax over
valid k only; out = attn @ values.

v5 design (low-rank separable tanh): tanh(a+b) ~= sum_r phi_r(a) psi_r(b)
with phi/psi from a Gaussian-weighted SVD of tanh on a grid (R=10 ranks,
end-to-end output err ~3e-3 in bf16).  This turns the whole O(B*Q*K*H)
tanh+score computation into one PE matmul with a 64*R=640 contraction:

  scores^T[k, q] = sum_m Bm[k, m] * A[q, m],   m = (r, h)
  A[q, m]  = Wv[h] * phi_r(qproj[q, h])        (host, bf16)
  Bm[k, m] = psi_r(kproj[k, h])                (host, bf16)

Device per batch (2 batches/core, data-parallel over B):
  - PE: per 128-k chunk, NPASS=5 matmuls (lhsT = Bm chunk [128m,128k],
    rhs = A [128m, 128q]) accumulate scores^T [128k, 128q] in psum
  - ACT: exp(psum) -> attn bf16 sbuf (k-major layout: no transpose needed)
  - PE: AV accumulation: lhsT = attn [128k, 128q], rhs = values_aug
    [128k, 258] -> [128q, 258] psum; col 256 is the ones column giving the
    softmax denominator; host zeroes rows >= valid_len so no masking needed
  - DVE: out = av[:, :256] * reciprocal(av[:, 256]); DMA store.

Batches are paired big+small by chunk count; compile shape (KA, KB) is the
max pair; shorter batches ship zero-padded chunks (scores 0 -> exp 1 ->
attn * zero values = 0, harmless).
"""

import sys

for _p in ("/opt/trn_rl_repo",):
    if _p not in sys.path:
        sys.path.append(_p)

import numpy as np
import ml_dtypes

import concourse.bass as bass  # noqa: F401
import concourse.tile as tile
from concourse import bacc, mybir
from concourse.bass_utils import run_bass_kernel_spmd

F32 = mybir.dt.float32
BF16 = mybir.dt.bfloat16
BF = ml_dtypes.bfloat16

B, Q, K, D, H, V = 16, 128, 1024, 256, 64, 256
VW = 258          # 256 values + ones column + pad
NCORES = 8
R = 10            # separable rank
HR = H * R        # contraction size
NPASS = HR // 128  # matmul passes per score chunk

_cache = {}
_basis_cache = {}


def _basis(sig):
    """Gaussian-weighted SVD of tanh(a+b) on a grid -> phi, psi [n, R]."""
    key = round(float(sig), 2)
    if key in _basis_cache:
        return _basis_cache[key]
    L = max(6.5, 5.5 * sig)
    n = 1301
    a = np.linspace(-L, L, n)
    p = np.exp(-a * a / (2.0 * sig * sig))
    w = np.sqrt(np.maximum(p, 3e-3))
    G = np.tanh(a[:, None] + a[None, :])
    M = w[:, None] * G * w[None, :]
    U, S, Vt = np.linalg.svd(M)
    phi = (U[:, :R] * np.sqrt(S[:R])) / w[:, None]
    psi = (Vt[:R].T * np.sqrt(S[:R])) / w[:, None]
    _basis_cache[key] = (L, n, a, phi, psi)
    return _basis_cache[key]


def _interp(x, L, n, F):
    idx = (np.clip(x, -L, L) + L) / (2 * L) * (n - 1)
    i0 = np.clip(idx.astype(np.int64), 0, n - 2)
    fr = (idx - i0)[..., None]
    return F[i0] * (1.0 - fr) + F[i0 + 1] * fr


def _build(ka, kb, exp_shift):
    nc = bacc.Bacc("TRN2", target_bir_lowering=False, debug=False,
                   num_devices=NCORES)
    kcs = (ka, kb)

    A_d = [nc.dram_tensor(f"A{b}", [128, NPASS * 128], BF16,
                          kind="ExternalInput") for b in range(2)]
    Bm_d = [nc.dram_tensor(f"Bm{b}", [kcs[b], 128, NPASS * 128], BF16,
                           kind="ExternalInput") for b in range(2)]
    vA_d = [nc.dram_tensor(f"vA{b}", [kcs[b], 128, VW], BF16,
                           kind="ExternalInput") for b in range(2)]
    out_d = nc.dram_tensor("out", [2, 128, V], F32, kind="ExternalOutput")

    with tile.TileContext(nc) as tc:
        with (
            tc.tile_pool(name="sb_A", bufs=2) as sb_A,
            tc.tile_pool(name="sb_B", bufs=ka + kb) as sb_B,
            tc.tile_pool(name="sb_v", bufs=ka + kb) as sb_v,
            tc.tile_pool(name="sb_attn", bufs=3) as sb_attn,
            tc.tile_pool(name="sb_out", bufs=2) as sb_out,
            tc.tile_pool(name="ps_sc", bufs=3, space="PSUM") as ps_sc,
            tc.tile_pool(name="ps_av", bufs=2, space="PSUM") as ps_av,
        ):
            # issue all loads up front, in consumption order; Bm on the SP
            # queue, A + values on the ACT queue so the two HWDGE queues
            # stream in parallel
            at = []
            bt = {}
            vt = {}
            for b in range(2):
                a_sb = sb_A.tile([128, NPASS, 128], BF16, tag="a",
                                 name=f"a{b}")
                nc.scalar.dma_start(out=a_sb, in_=A_d[b].ap().rearrange(
                    "p (s q) -> p s q", s=NPASS))
                at.append(a_sb)
                for c in range(kcs[b]):
                    bm = sb_B.tile([128, NPASS, 128], BF16, tag="bm",
                                   name=f"bm{b}_{c}")
                    nc.sync.dma_start(out=bm, in_=Bm_d[b].ap()[c].rearrange(
                        "p (s k) -> p s k", s=NPASS))
                    bt[(b, c)] = bm
                    vv = sb_v.tile([128, VW], BF16, tag="v", name=f"v{b}_{c}")
                    nc.scalar.dma_start(out=vv, in_=vA_d[b].ap()[c])
                    vt[(b, c)] = vv

            for b in range(2):
                kc = kcs[b]
                av = ps_av.tile([128, VW], F32, tag="av", name=f"av{b}")
                for c in range(kc):
                    sc = ps_sc.tile([128, 128], F32, tag="sc",
                                    name=f"sc{b}_{c}")
                    for p in range(NPASS):
                        nc.tensor.matmul(
                            sc, bt[(b, c)][:, p, :], at[b][:, p, :],
                            start=(p == 0), stop=(p == NPASS - 1))
                    attn = sb_attn.tile([128, 128], BF16, tag="attn",
                                        name=f"attn{b}_{c}")
                    nc.scalar.activation(
                        attn, sc, mybir.ActivationFunctionType.Exp,
                        bias=-exp_shift)
                    nc.tensor.matmul(
                        av, attn, vt[(b, c)],
                        start=(c == 0), stop=(c == kc - 1))
                rcp = sb_out.tile([128, 1], F32, tag="rcp", name=f"rcp{b}")
                nc.vector.reciprocal(rcp, av[:, V:V + 1])
                outt = sb_out.tile([128, V], F32, tag="out", name=f"out{b}")
                nc.vector.tensor_scalar_mul(outt, av[:, 0:V], rcp)
                nc.sync.dma_start(out=out_d.ap()[b], in_=outt)

    nc.compile()
    return nc


def _prep(queries, keys, values, valid_lens, Wq, Wk, Wv):
    vl = [int(x) for x in np.asarray(valid_lens).reshape(-1)]
    assert len(vl) == B

    qf = np.asarray(queries, np.float32)
    kf = np.asarray(keys, np.float32)
    qproj = np.einsum('bqd,hd->bqh', qf, np.asarray(Wq, np.float32))
    kproj = np.einsum('bkd,hd->bkh', kf, np.asarray(Wk, np.float32))
    wv = np.asarray(Wv, np.float32).reshape(-1)                  # [H]

    sig = float(np.sqrt((qproj.var() + kproj.var()) / 2.0))
    sig = min(max(sig, 0.3), 4.0)
    L, n, _, phi, psi = _basis(sig)

    # A[b, q, m], m = r*64 + h, weighted by Wv
    phq = _interp(qproj, L, n, phi)          # [B, Q, H, R]
    Am = np.transpose(phq * wv[None, None, :, None],
                      (0, 3, 2, 1)).reshape(B, HR, Q)  # [b, m, q]
    psk = _interp(kproj, L, n, psi)          # [B, K, H, R]
    Bmm = np.transpose(psk, (0, 3, 2, 1)).reshape(B, HR, K)  # [b, m, k]

    bound = float(np.abs(wv).sum())
    exp_shift = max(0.0, bound - 30.0)

    # pair batches big+small by chunk count to minimize (KA, KB)
    kc_b = [(v + 127) // 128 for v in vl]
    order = sorted(range(B), key=lambda b: -kc_b[b])
    pairs = [(order[i], order[B - 1 - i]) for i in range(NCORES)]
    ka = max(kc_b[p[0]] for p in pairs)
    kb = max(kc_b[p[1]] for p in pairs)

    va = np.zeros((B, K, VW), BF)
    va[:, :, :V] = np.asarray(values, BF)
    va[:, :, V] = BF(1.0)
    for b in range(B):
        va[b, vl[b]:, :] = 0

    in_maps = []
    assignment = []
    for c in range(NCORES):
        m = {}
        for slot, (bb, cap) in enumerate(zip(pairs[c], (ka, kb))):
            kc = kc_b[bb]
            # A: [128i, NPASS*128q], element (i, s, q) = Am[b, s*128+i, q]
            a_arr = Am[bb].reshape(NPASS, 128, Q).transpose(1, 0, 2)
            m[f"A{slot}"] = np.ascontiguousarray(
                a_arr.reshape(128, NPASS * Q)).astype(BF)
            # Bm: [cap, 128i, NPASS*128k], (c, i, s, k') =
            #     Bmm[b, s*128+i, c*128+k']; zero-pad chunks >= kc
            b_arr = np.zeros((cap, 128, NPASS * 128), BF)
            src = Bmm[bb].reshape(NPASS, 128, 8, 128)  # [s, i, c, k']
            for cc in range(kc):
                b_arr[cc] = np.ascontiguousarray(
                    src[:, :, cc, :].transpose(1, 0, 2).reshape(
                        128, NPASS * 128)).astype(BF)
            m[f"Bm{slot}"] = b_arr
            # values: [cap, 128k', VW], zero-padded
            v_arr = np.zeros((cap, 128, VW), BF)
            v_arr[:kc] = va[bb].reshape(8, 128, VW)[:kc]
            m[f"vA{slot}"] = v_arr
        in_maps.append(m)
        assignment.append(pairs[c])
    return ka, kb, exp_shift, in_maps, assignment


def kernel(queries, keys, values, valid_lens, Wq, Wk, Wv):
    ka, kb, exp_shift, in_maps, assignment = _prep(
        queries, keys, values, valid_lens, Wq, Wk, Wv)
    key = (ka, kb, round(exp_shift, 3))
    if key not in _cache:
        _cache[key] = _build(ka, kb, exp_shift)
    nc = _cache[key]
    res = run_bass_kernel_spmd(nc, in_maps, list(range(NCORES)))
    out = np.zeros((B, Q, V), np.float32)
    for c in range(NCORES):
        o = res.results[c]["out"]           # [2, 128, V]
        for slot, bb in enumerate(assignment[c]):
            out[bb] = o[slot]
    return out


if __name__ == "__main__":
    from concourse.bass_interp import CoreSim

    rng = np.random.default_rng(0)
    queries = rng.standard_normal((B, Q, D), np.float32)
    keys = rng.standard_normal((B, K, D), np.float32)
    values = rng.standard_normal((B, K, V), np.float32)
    valid_lens = rng.integers(1, K + 1, (B,)).astype(np.int64)
    Wq = (rng.standard_normal((H, D), np.float32) / np.sqrt(D)).astype(np.float32)
    Wk = (rng.standard_normal((H, D), np.float32) / np.sqrt(D)).astype(np.float32)
    Wv = (rng.standard_normal((1, H), np.float32) / np.sqrt(H)).astype(np.float32)

    ka, kb, exp_shift, in_maps, assignment = _prep(
        queries, keys, values, valid_lens, Wq, Wk, Wv)
    print("ka, kb:", ka, kb, "exp_shift:", exp_shift)
    nc = _build(ka, kb, exp_shift)
    print("built+compiled")

    sim = CoreSim(nc, trace=False)
    for name, arr in in_maps[0].items():
        sim.tensor(name)[:] = arr
    sim.simulate()
    got = np.array(sim.tensor("out"))

    q = queries @ Wq.T
    k = keys @ Wk.T
    worst = 0.0
    for slot, b in enumerate(assignment[0]):
        feats = np.tanh(q[b][:, None, :] + k[b][None, :, :])
        scores = feats @ Wv[0]
        vlb = int(valid_lens[b])
        scores[:, vlb:] = -1e6
        e = np.exp(scores - scores.max(-1, keepdims=True))
        attn = e / e.sum(-1, keepdims=True)
        exp_out = attn @ values[b]
        gj = got[slot]
        err = np.abs(gj - exp_out)
        rel = err.max() / np.abs(exp_out).max()
        worst = max(worst, rel)
        print(f"slot {slot} (b={b}, vl={vlb}): absmax-rel err {rel:.3e}")
    print("worst:", worst)


# revision 3
# speedup vs baseline: 1.0993x; 1.0993x over previous
"""Additive attention (B=16, Q=128, K=1024, D=256, H=64) on 8 trn2 NeuronCores.

scores[b,q,k] = sum_h Wv[h] * tanh(qproj[b,q,h] + kproj[b,k,h]); softmax over
valid k only; out = attn @ values.

v5 design (low-rank separable tanh): tanh(a+b) ~= sum_r phi_r(a) psi_r(b)
with phi/psi from a Gaussian-weighted SVD of tanh on a grid (R=10 ranks,
end-to-end output err ~3e-3).  This turns the whole O(B*Q*K*H) tanh+score
computation into one PE matmul with a 64*R=640 contraction:

  scores^T[k, q] = sum_m Bm[k, m] * A[q, m],   m = (r, h)
  A[q, m]  = Wv[h] * phi_r(qproj[q, h])        (host)
  Bm[k, m] = psi_r(kproj[k, h])                (host)

Ranks 0-3 (passes 0-1) ship in bf16; ranks 4-9 (passes 2-4) have tiny
amplitude and ship in fp8e4m3 (halves those bytes; output err unchanged at
~3e-3).  DMAs are few and large (128 descriptors each, partition-major
dram layouts) because each DMA trigger costs ~600ns on the issuing queue.

Device per batch (2 batches/core, data-parallel over B):
  - PE: per 128-k chunk, 5 matmuls (lhsT = Bm chunk [128m,128k], rhs =
    A [128m, 128q]) accumulate scores^T [128k, 128q] in psum
  - ACT: exp(psum) -> attn bf16 sbuf (k-major layout: no transpose needed)
  - PE: AV accumulation: lhsT = attn [128k, 128q], rhs = values_aug
    [128k, 258] -> [128q, 258] psum; col 256 is the ones column giving the
    softmax denominator; host zeroes rows >= valid_len so no masking needed
  - DVE: out = av[:, :256] * reciprocal(av[:, 256]); DMA store.

Batches are paired big+small by chunk count; compile shape (KA, KB) is the
max pair; shorter batches ship zero-padded chunks (scores 0 -> exp 1 ->
attn * zero values = 0, harmless).
"""

import sys

for _p in ("/opt/trn_rl_repo",):
    if _p not in sys.path:
        sys.path.append(_p)

import numpy as np
import ml_dtypes

import concourse.bass as bass  # noqa: F401
import concourse.tile as tile
from concourse import bacc, mybir
from concourse.bass_utils import run_bass_kernel_spmd

F32 = mybir.dt.float32
BF16 = mybir.dt.bfloat16
FP8 = mybir.dt.float8e4
BF = ml_dtypes.bfloat16
F8 = ml_dtypes.float8_e4m3

B, Q, K, D, H, V = 16, 128, 1024, 256, 64, 256
VW = 258          # 256 values + ones column + pad
NCORES = 8
R = 10            # separable rank
HR = H * R        # contraction size
NPASS = HR // 128  # matmul passes per score chunk
NBF = 2           # passes 0..NBF-1 in bf16, rest fp8
NF8 = NPASS - NBF
HEAD = 2          # slot-A chunks in the early Bm piece

_cache = {}
_basis_cache = {}


def _basis(sig):
    """Gaussian-weighted SVD of tanh(a+b) on a grid -> phi, psi [n, R]."""
    key = round(float(sig), 2)
    if key in _basis_cache:
        return _basis_cache[key]
    L = max(6.5, 5.5 * sig)
    n = 1301
    a = np.linspace(-L, L, n)
    p = np.exp(-a * a / (2.0 * sig * sig))
    w = np.sqrt(np.maximum(p, 3e-3))
    G = np.tanh(a[:, None] + a[None, :])
    M = w[:, None] * G * w[None, :]
    U, S, Vt = np.linalg.svd(M)
    phi = (U[:, :R] * np.sqrt(S[:R])) / w[:, None]
    psi = (Vt[:R].T * np.sqrt(S[:R])) / w[:, None]
    _basis_cache[key] = (L, n, a, phi, psi)
    return _basis_cache[key]


def _interp(x, L, n, F):
    idx = (np.clip(x, -L, L) + L) / (2 * L) * (n - 1)
    i0 = np.clip(idx.astype(np.int64), 0, n - 2)
    fr = (idx - i0)[..., None]
    return F[i0] * (1.0 - fr) + F[i0 + 1] * fr


def _build(ka, kb, exp_shift):
    nc = bacc.Bacc("TRN2", target_bir_lowering=False, debug=False,
                   num_devices=NCORES)
    kcs = (ka, kb)

    Ab_d = [nc.dram_tensor(f"Ab{b}", [128, NBF * 128], BF16,
                           kind="ExternalInput") for b in range(2)]
    A8_d = [nc.dram_tensor(f"A8{b}", [128, NF8 * 128], FP8,
                           kind="ExternalInput") for b in range(2)]
    Bb_d = [nc.dram_tensor(f"Bb{b}", [128, kcs[b], NBF, 128], BF16,
                           kind="ExternalInput") for b in range(2)]
    B8_d = [nc.dram_tensor(f"B8{b}", [128, kcs[b], NF8, 128], FP8,
                           kind="ExternalInput") for b in range(2)]
    vA_d = [nc.dram_tensor(f"vA{b}", [128, kcs[b], VW], BF16,
                           kind="ExternalInput") for b in range(2)]
    out_d = nc.dram_tensor("out", [2, 128, V], F32, kind="ExternalOutput")

    with tile.TileContext(nc) as tc:
        with (
            tc.tile_pool(name="sb_A", bufs=4) as sb_A,
            tc.tile_pool(name="sb_B", bufs=6) as sb_B,
            tc.tile_pool(name="sb_v", bufs=2) as sb_v,
            tc.tile_pool(name="sb_attn", bufs=3) as sb_attn,
            tc.tile_pool(name="sb_out", bufs=2) as sb_out,
            tc.tile_pool(name="ps_sc", bufs=3, space="PSUM") as ps_sc,
            tc.tile_pool(name="ps_av", bufs=2, space="PSUM") as ps_av,
        ):
            # Few, large DMAs (each ~600ns of trigger time on its queue).
            # scalar queue: A + values; sync queue: the Bm stream.  Slot A's
            # Bm is split head/tail so PE can start early.
            ab, a8, vt = [], [], []
            for b in range(2):
                t = sb_A.tile([128, NBF, 128], BF16, tag="ab", name=f"ab{b}")
                nc.scalar.dma_start(out=t, in_=Ab_d[b].ap().rearrange(
                    "p (s q) -> p s q", s=NBF))
                ab.append(t)
                t = sb_A.tile([128, NF8, 128], FP8, tag="a8", name=f"a8{b}")
                nc.scalar.dma_start(out=t, in_=A8_d[b].ap().rearrange(
                    "p (s q) -> p s q", s=NF8))
                a8.append(t)
            bb = {}
            b8 = {}
            for b, (lo, hi) in ((0, (0, HEAD)), (0, (HEAD, ka)),
                                (1, (0, kb))):
                t = sb_B.tile([128, hi - lo, NBF, 128], BF16, tag="bb",
                              name=f"bb{b}_{lo}")
                nc.sync.dma_start(out=t, in_=Bb_d[b].ap()[:, lo:hi])
                for c in range(lo, hi):
                    bb[(b, c)] = t[:, c - lo]
                t = sb_B.tile([128, hi - lo, NF8, 128], FP8, tag="b8",
                              name=f"b8{b}_{lo}")
                nc.sync.dma_start(out=t, in_=B8_d[b].ap()[:, lo:hi])
                for c in range(lo, hi):
                    b8[(b, c)] = t[:, c - lo]
            for b in range(2):
                t = sb_v.tile([128, kcs[b], VW], BF16, tag="v", name=f"v{b}")
                nc.scalar.dma_start(out=t, in_=vA_d[b].ap())
                vt.append(t)

            for b in range(2):
                kc = kcs[b]
                av = ps_av.tile([128, VW], F32, tag="av", name=f"av{b}")
                for c in range(kc):
                    sc = ps_sc.tile([128, 128], F32, tag="sc",
                                    name=f"sc{b}_{c}")
                    for p in range(NBF):
                        nc.tensor.matmul(
                            sc, bb[(b, c)][:, p, :], ab[b][:, p, :],
                            start=(p == 0), stop=False)
                    for p in range(NF8):
                        nc.tensor.matmul(
                            sc, b8[(b, c)][:, p, :], a8[b][:, p, :],
                            start=False, stop=(p == NF8 - 1))
                    attn = sb_attn.tile([128, 128], BF16, tag="attn",
                                        name=f"attn{b}_{c}")
                    nc.scalar.activation(
                        attn, sc, mybir.ActivationFunctionType.Exp,
                        bias=-exp_shift)
                    nc.tensor.matmul(
                        av, attn, vt[b][:, c, :],
                        start=(c == 0), stop=(c == kc - 1))
                rcp = sb_out.tile([128, 1], F32, tag="rcp", name=f"rcp{b}")
                nc.vector.reciprocal(rcp, av[:, V:V + 1])
                outt = sb_out.tile([128, V], F32, tag="out", name=f"out{b}")
                nc.vector.tensor_scalar_mul(outt, av[:, 0:V], rcp)
                nc.sync.dma_start(out=out_d.ap()[b], in_=outt)

    nc.compile()
    return nc


def _prep(queries, keys, values, valid_lens, Wq, Wk, Wv):
    vl = [int(x) for x in np.asarray(valid_lens).reshape(-1)]
    assert len(vl) == B

    qf = np.asarray(queries, np.float32)
    kf = np.asarray(keys, np.float32)
    qproj = np.einsum('bqd,hd->bqh', qf, np.asarray(Wq, np.float32))
    kproj = np.einsum('bkd,hd->bkh', kf, np.asarray(Wk, np.float32))
    wv = np.asarray(Wv, np.float32).reshape(-1)                  # [H]

    sig = float(np.sqrt((qproj.var() + kproj.var()) / 2.0))
    sig = min(max(sig, 0.3), 4.0)
    L, n, _, phi, psi = _basis(sig)

    # A[b, m, q], Bm[b, m, k]; m = r*64 + h, A weighted by Wv
    phq = _interp(qproj, L, n, phi)          # [B, Q, H, R]
    Am = np.transpose(phq * wv[None, None, :, None],
                      (0, 3, 2, 1)).reshape(B, HR, Q)
    psk = _interp(kproj, L, n, psi)          # [B, K, H, R]
    Bmm = np.transpose(psk, (0, 3, 2, 1)).reshape(B, HR, K)

    bound = float(np.abs(wv).sum())
    exp_shift = max(0.0, bound - 30.0)

    # pair batches big+small by chunk count to minimize (KA, KB)
    kc_b = [(x + 127) // 128 for x in vl]
    order = sorted(range(B), key=lambda b: -kc_b[b])
    pairs = [(order[i], order[B - 1 - i]) for i in range(NCORES)]
    ka = max(kc_b[p[0]] for p in pairs)
    kb = max(kc_b[p[1]] for p in pairs)

    va = np.zeros((B, K, VW), BF)
    va[:, :, :V] = np.asarray(values, BF)
    va[:, :, V] = BF(1.0)
    for b in range(B):
        va[b, vl[b]:, :] = 0

    in_maps = []
    assignment = []
    for c in range(NCORES):
        m = {}
        for slot, (bbn, cap) in enumerate(zip(pairs[c], (ka, kb))):
            kc = kc_b[bbn]
            # A: [128i, s, 128q], element (i, s, q) = Am[b, s*128+i, q]
            a_arr = Am[bbn].reshape(NPASS, 128, Q).transpose(1, 0, 2)
            m[f"Ab{slot}"] = np.ascontiguousarray(
                a_arr[:, :NBF].reshape(128, NBF * Q)).astype(BF)
            m[f"A8{slot}"] = np.ascontiguousarray(
                a_arr[:, NBF:].reshape(128, NF8 * Q)).astype(F8)
            # Bm: [128i, cap, s, 128k'], (i, c, s, k') =
            #     Bmm[b, s*128+i, c*128+k']; zero-pad chunks >= kc
            src = Bmm[bbn].reshape(NPASS, 128, 8, 128)  # [s, i, c, k']
            b_arr = np.zeros((128, cap, NPASS, 128), np.float32)
            b_arr[:, :kc] = src.transpose(1, 2, 0, 3)[:, :kc]
            m[f"Bb{slot}"] = np.ascontiguousarray(b_arr[:, :, :NBF]).astype(BF)
            m[f"B8{slot}"] = np.ascontiguousarray(b_arr[:, :, NBF:]).astype(F8)
            # values: [128k', cap, VW], zero-padded
            v_arr = np.zeros((128, cap, VW), BF)
            v_arr[:, :kc] = va[bbn].reshape(8, 128, VW)[:kc].transpose(1, 0, 2)
            m[f"vA{slot}"] = v_arr
        in_maps.append(m)
        assignment.append(pairs[c])
    return ka, kb, exp_shift, in_maps, assignment


def kernel(queries, keys, values, valid_lens, Wq, Wk, Wv):
    ka, kb, exp_shift, in_maps, assignment = _prep(
        queries, keys, values, valid_lens, Wq, Wk, Wv)
    key = (ka, kb, round(exp_shift, 3))
    if key not in _cache:
        _cache[key] = _build(ka, kb, exp_shift)
    nc = _cache[key]
    res = run_bass_kernel_spmd(nc, in_maps, list(range(NCORES)))
    out = np.zeros((B, Q, V), np.float32)
    for c in range(NCORES):
        o = res.results[c]["out"]           # [2, 128, V]
        for slot, bbn in enumerate(assignment[c]):
            out[bbn] = o[slot]
    return out


if __name__ == "__main__":
    from concourse.bass_interp import CoreSim

    rng = np.random.default_rng(0)
    queries = rng.standard_normal((B, Q, D), np.float32)
    keys = rng.standard_normal((B, K, D), np.float32)
    values = rng.standard_normal((B, K, V), np.float32)
    valid_lens = rng.integers(1, K + 1, (B,)).astype(np.int64)
    Wq = (rng.standard_normal((H, D), np.float32) / np.sqrt(D)).astype(np.float32)
    Wk = (rng.standard_normal((H, D), np.float32) / np.sqrt(D)).astype(np.float32)
    Wv = (rng.standard_normal((1, H), np.float32) / np.sqrt(H)).astype(np.float32)

    ka, kb, exp_shift, in_maps, assignment = _prep(
        queries, keys, values, valid_lens, Wq, Wk, Wv)
    print("ka, kb:", ka, kb, "exp_shift:", exp_shift)
    nc = _build(ka, kb, exp_shift)
    print("built+compiled")

    sim = CoreSim(nc, trace=False)
    for name, arr in in_maps[0].items():
        sim.tensor(name)[:] = arr
    sim.simulate()
    got = np.array(sim.tensor("out"))

    q = queries @ Wq.T
    k = keys @ Wk.T
    worst = 0.0
    for slot, b in enumerate(assignment[0]):
        feats = np.tanh(q[b][:, None, :] + k[b][None, :, :])
        scores = feats @ Wv[0]
        vlb = int(valid_lens[b])
        scores[:, vlb:] = -1e6
        e = np.exp(scores - scores.max(-1, keepdims=True))
        attn = e / e.sum(-1, keepdims=True)
        exp_out = attn @ values[b]
        gj = got[slot]
        err = np.abs(gj - exp_out)
        rel = err.max() / np.abs(exp_out).max()
        worst = max(worst, rel)
        print(f"slot {slot} (b={b}, vl={vlb}): absmax-rel err {rel:.3e}")
    print("worst:", worst)


# revision 5
# speedup vs baseline: 1.1068x; 1.0069x over previous
"""Additive attention (B=16, Q=128, K=1024, D=256, H=64) on 8 trn2 NeuronCores.

scores[b,q,k] = sum_h Wv[h] * tanh(qproj[b,q,h] + kproj[b,k,h]); softmax over
valid k only; out = attn @ values.

v5 design (low-rank separable tanh): tanh(a+b) ~= sum_r phi_r(a) psi_r(b)
with phi/psi from a Gaussian-weighted SVD of tanh on a grid (R=10 ranks,
end-to-end output err ~3e-3).  This turns the whole O(B*Q*K*H) tanh+score
computation into one PE matmul with a 64*R=640 contraction:

  scores^T[k, q] = sum_m Bm[k, m] * A[q, m],   m = (r, h)
  A[q, m]  = Wv[h] * phi_r(qproj[q, h])        (host)
  Bm[k, m] = psi_r(kproj[k, h])                (host)

Ranks 0-3 (passes 0-1) ship in bf16; ranks 4-9 (passes 2-4) have tiny
amplitude and ship in fp8e4m3 (halves those bytes; output err unchanged at
~3e-3).  DMAs are few and large (128 descriptors each, partition-major
dram layouts) because each DMA trigger costs ~600ns on the issuing queue.

Device per batch (2 batches/core, data-parallel over B):
  - PE: per 128-k chunk, 5 matmuls (lhsT = Bm chunk [128m,128k], rhs =
    A [128m, 128q]) accumulate scores^T [128k, 128q] in psum
  - ACT: exp(psum) -> attn bf16 sbuf (k-major layout: no transpose needed)
  - PE: AV accumulation: lhsT = attn [128k, 128q], rhs = values_aug
    [128k, 258] -> [128q, 258] psum; col 256 is the ones column giving the
    softmax denominator; host zeroes rows >= valid_len so no masking needed
  - DVE: out = av[:, :256] * reciprocal(av[:, 256]); DMA store.

Batches are paired big+small by chunk count; compile shape (KA, KB) is the
max pair; shorter batches ship zero-padded chunks (scores 0 -> exp 1 ->
attn * zero values = 0, harmless).
"""

import sys

for _p in ("/opt/trn_rl_repo",):
    if _p not in sys.path:
        sys.path.append(_p)

import numpy as np
import ml_dtypes

import concourse.bass as bass  # noqa: F401
import concourse.tile as tile
from concourse import bacc, mybir
from concourse.bass_utils import run_bass_kernel_spmd

F32 = mybir.dt.float32
BF16 = mybir.dt.bfloat16
FP8 = mybir.dt.float8e4
BF = ml_dtypes.bfloat16
F8 = ml_dtypes.float8_e4m3

B, Q, K, D, H, V = 16, 128, 1024, 256, 64, 256
VW = 258          # 256 values + ones column + pad
NCORES = 8
R = 10            # separable rank
HR = H * R        # contraction size
NPASS = HR // 128  # matmul passes per score chunk
NBF = 2           # passes 0..NBF-1 in bf16, rest fp8
NF8 = NPASS - NBF
HEAD = 2          # slot-A chunks in the early Bm piece

_cache = {}
_basis_cache = {}


def _basis(sig):
    """Gaussian-weighted SVD of tanh(a+b) on a grid -> phi, psi [n, R]."""
    key = round(float(sig), 2)
    if key in _basis_cache:
        return _basis_cache[key]
    L = max(6.5, 5.5 * sig)
    n = 1301
    a = np.linspace(-L, L, n)
    p = np.exp(-a * a / (2.0 * sig * sig))
    w = np.sqrt(np.maximum(p, 3e-3))
    G = np.tanh(a[:, None] + a[None, :])
    M = w[:, None] * G * w[None, :]
    U, S, Vt = np.linalg.svd(M)
    phi = (U[:, :R] * np.sqrt(S[:R])) / w[:, None]
    psi = (Vt[:R].T * np.sqrt(S[:R])) / w[:, None]
    _basis_cache[key] = (L, n, a, phi, psi)
    return _basis_cache[key]


def _interp(x, L, n, F):
    idx = (np.clip(x, -L, L) + L) / (2 * L) * (n - 1)
    i0 = np.clip(idx.astype(np.int64), 0, n - 2)
    fr = (idx - i0)[..., None]
    return F[i0] * (1.0 - fr) + F[i0 + 1] * fr


def _build(ka, kb, exp_shift):
    nc = bacc.Bacc("TRN2", target_bir_lowering=False, debug=False,
                   num_devices=NCORES)
    kcs = (ka, kb)

    Ab_d = [nc.dram_tensor(f"Ab{b}", [128, NBF * 128], BF16,
                           kind="ExternalInput") for b in range(2)]
    A8_d = [nc.dram_tensor(f"A8{b}", [128, NF8 * 128], FP8,
                           kind="ExternalInput") for b in range(2)]
    Bb_d = [nc.dram_tensor(f"Bb{b}", [128, kcs[b], NBF, 128], BF16,
                           kind="ExternalInput") for b in range(2)]
    B8_d = [nc.dram_tensor(f"B8{b}", [128, kcs[b], NF8, 128], FP8,
                           kind="ExternalInput") for b in range(2)]
    vA_d = [nc.dram_tensor(f"vA{b}", [128, kcs[b], VW], BF16,
                           kind="ExternalInput") for b in range(2)]
    out_d = nc.dram_tensor("out", [2, 128, V], F32, kind="ExternalOutput")

    with tile.TileContext(nc) as tc:
        with (
            tc.tile_pool(name="sb_A", bufs=4) as sb_A,
            tc.tile_pool(name="sb_B", bufs=6) as sb_B,
            tc.tile_pool(name="sb_v", bufs=2) as sb_v,
            tc.tile_pool(name="sb_attn", bufs=4) as sb_attn,
            tc.tile_pool(name="sb_out", bufs=2) as sb_out,
            tc.tile_pool(name="ps_sc", bufs=4, space="PSUM") as ps_sc,
            tc.tile_pool(name="ps_av", bufs=2, space="PSUM") as ps_av,
        ):
            # Few, large DMAs (each ~600ns of trigger time on its queue).
            # scalar queue: A + values; sync queue: the Bm stream.  Slot A's
            # Bm is split head/tail so PE can start early.
            ab, a8, vt = [], [], []
            for b in range(2):
                t = sb_A.tile([128, NBF, 128], BF16, tag="ab", name=f"ab{b}")
                nc.scalar.dma_start(out=t, in_=Ab_d[b].ap().rearrange(
                    "p (s q) -> p s q", s=NBF))
                ab.append(t)
                t = sb_A.tile([128, NF8, 128], FP8, tag="a8", name=f"a8{b}")
                nc.scalar.dma_start(out=t, in_=A8_d[b].ap().rearrange(
                    "p (s q) -> p s q", s=NF8))
                a8.append(t)
            bb = {}
            b8 = {}
            for b, (lo, hi) in ((0, (0, HEAD)), (0, (HEAD, ka)),
                                (1, (0, kb))):
                t = sb_B.tile([128, hi - lo, NBF, 128], BF16, tag="bb",
                              name=f"bb{b}_{lo}")
                nc.sync.dma_start(out=t, in_=Bb_d[b].ap()[:, lo:hi])
                for c in range(lo, hi):
                    bb[(b, c)] = t[:, c - lo]
                t = sb_B.tile([128, hi - lo, NF8, 128], FP8, tag="b8",
                              name=f"b8{b}_{lo}")
                nc.sync.dma_start(out=t, in_=B8_d[b].ap()[:, lo:hi])
                for c in range(lo, hi):
                    b8[(b, c)] = t[:, c - lo]
            for b in range(2):
                t = sb_v.tile([128, kcs[b], VW], BF16, tag="v", name=f"v{b}")
                nc.scalar.dma_start(out=t, in_=vA_d[b].ap())
                vt.append(t)

            # software-pipelined: scores+exp of job i run while AV of job
            # i-DELAY consumes an older chunk's attn, so the PE never waits
            # on the ACT exp latency.
            DELAY = 2
            jobs = [(b, c) for b in range(2) for c in range(kcs[b])]
            avs = {b: ps_av.tile([128, VW], F32, tag="av", name=f"av{b}")
                   for b in range(2)}
            attns = {}

            def do_av(b, c):
                nc.tensor.matmul(
                    avs[b], attns.pop((b, c)), vt[b][:, c, :],
                    start=(c == 0), stop=(c == kcs[b] - 1))
                if c == kcs[b] - 1:
                    rcp = sb_out.tile([128, 1], F32, tag="rcp",
                                      name=f"rcp{b}")
                    nc.vector.reciprocal(rcp, avs[b][:, V:V + 1])
                    outt = sb_out.tile([128, V], F32, tag="out",
                                       name=f"out{b}")
                    nc.vector.tensor_scalar_mul(outt, avs[b][:, 0:V], rcp)
                    nc.sync.dma_start(out=out_d.ap()[b], in_=outt)

            for i, (b, c) in enumerate(jobs):
                sc = ps_sc.tile([128, 128], F32, tag="sc", name=f"sc{b}_{c}")
                for p in range(NBF):
                    nc.tensor.matmul(
                        sc, bb[(b, c)][:, p, :], ab[b][:, p, :],
                        start=(p == 0), stop=False)
                for p in range(NF8):
                    nc.tensor.matmul(
                        sc, b8[(b, c)][:, p, :], a8[b][:, p, :],
                        start=False, stop=(p == NF8 - 1))
                attn = sb_attn.tile([128, 128], BF16, tag="attn",
                                    name=f"attn{b}_{c}")
                nc.scalar.activation(
                    attn, sc, mybir.ActivationFunctionType.Exp,
                    bias=-exp_shift)
                attns[(b, c)] = attn
                if i >= DELAY:
                    do_av(*jobs[i - DELAY])
            for j in jobs[-DELAY:]:
                do_av(*j)

    nc.compile()
    return nc


def _prep(queries, keys, values, valid_lens, Wq, Wk, Wv):
    vl = [int(x) for x in np.asarray(valid_lens).reshape(-1)]
    assert len(vl) == B

    qf = np.asarray(queries, np.float32)
    kf = np.asarray(keys, np.float32)
    qproj = np.einsum('bqd,hd->bqh', qf, np.asarray(Wq, np.float32))
    kproj = np.einsum('bkd,hd->bkh', kf, np.asarray(Wk, np.float32))
    wv = np.asarray(Wv, np.float32).reshape(-1)                  # [H]

    sig = float(np.sqrt((qproj.var() + kproj.var()) / 2.0))
    sig = min(max(sig, 0.3), 4.0)
    L, n, _, phi, psi = _basis(sig)

    # A[b, m, q], Bm[b, m, k]; m = r*64 + h, A weighted by Wv
    phq = _interp(qproj, L, n, phi)          # [B, Q, H, R]
    Am = np.transpose(phq * wv[None, None, :, None],
                      (0, 3, 2, 1)).reshape(B, HR, Q)
    psk = _interp(kproj, L, n, psi)          # [B, K, H, R]
    Bmm = np.transpose(psk, (0, 3, 2, 1)).reshape(B, HR, K)

    bound = float(np.abs(wv).sum())
    exp_shift = max(0.0, bound - 30.0)

    # pair batches big+small by chunk count to minimize (KA, KB)
    kc_b = [(x + 127) // 128 for x in vl]
    order = sorted(range(B), key=lambda b: -kc_b[b])
    pairs = [(order[i], order[B - 1 - i]) for i in range(NCORES)]
    ka = max(kc_b[p[0]] for p in pairs)
    kb = max(kc_b[p[1]] for p in pairs)

    va = np.zeros((B, K, VW), BF)
    va[:, :, :V] = np.asarray(values, BF)
    va[:, :, V] = BF(1.0)
    for b in range(B):
        va[b, vl[b]:, :] = 0

    in_maps = []
    assignment = []
    for c in range(NCORES):
        m = {}
        for slot, (bbn, cap) in enumerate(zip(pairs[c], (ka, kb))):
            kc = kc_b[bbn]
            # A: [128i, s, 128q], element (i, s, q) = Am[b, s*128+i, q]
            a_arr = Am[bbn].reshape(NPASS, 128, Q).transpose(1, 0, 2)
            m[f"Ab{slot}"] = np.ascontiguousarray(
                a_arr[:, :NBF].reshape(128, NBF * Q)).astype(BF)
            m[f"A8{slot}"] = np.ascontiguousarray(
                a_arr[:, NBF:].reshape(128, NF8 * Q)).astype(F8)
            # Bm: [128i, cap, s, 128k'], (i, c, s, k') =
            #     Bmm[b, s*128+i, c*128+k']; zero-pad chunks >= kc
            src = Bmm[bbn].reshape(NPASS, 128, 8, 128)  # [s, i, c, k']
            b_arr = np.zeros((128, cap, NPASS, 128), np.float32)
            b_arr[:, :kc] = src.transpose(1, 2, 0, 3)[:, :kc]
            m[f"Bb{slot}"] = np.ascontiguousarray(b_arr[:, :, :NBF]).astype(BF)
            m[f"B8{slot}"] = np.ascontiguousarray(b_arr[:, :, NBF:]).astype(F8)
            # values: [128k', cap, VW], zero-padded
            v_arr = np.zeros((128, cap, VW), BF)
            v_arr[:, :kc] = va[bbn].reshape(8, 128, VW)[:kc].transpose(1, 0, 2)
            m[f"vA{slot}"] = v_arr
        in_maps.append(m)
        assignment.append(pairs[c])
    return ka, kb, exp_shift, in_maps, assignment


def kernel(queries, keys, values, valid_lens, Wq, Wk, Wv):
    ka, kb, exp_shift, in_maps, assignment = _prep(
        queries, keys, values, valid_lens, Wq, Wk, Wv)
    key = (ka, kb, round(exp_shift, 3))
    if key not in _cache:
        _cache[key] = _build(ka, kb, exp_shift)
    nc = _cache[key]
    res = run_bass_kernel_spmd(nc, in_maps, list(range(NCORES)))
    out = np.zeros((B, Q, V), np.float32)
    for c in range(NCORES):
        o = res.results[c]["out"]           # [2, 128, V]
        for slot, bbn in enumerate(assignment[c]):
            out[bbn] = o[slot]
    return out


if __name__ == "__main__":
    from concourse.bass_interp import CoreSim

    rng = np.random.default_rng(0)
    queries = rng.standard_normal((B, Q, D), np.float32)
    keys = rng.standard_normal((B, K, D), np.float32)
    values = rng.standard_normal((B, K, V), np.float32)
    valid_lens = rng.integers(1, K + 1, (B,)).astype(np.int64)
    Wq = (rng.standard_normal((H, D), np.float32) / np.sqrt(D)).astype(np.float32)
    Wk = (rng.standard_normal((H, D), np.float32) / np.sqrt(D)).astype(np.float32)
    Wv = (rng.standard_normal((1, H), np.float32) / np.sqrt(H)).astype(np.float32)

    ka, kb, exp_shift, in_maps, assignment = _prep(
        queries, keys, values, valid_lens, Wq, Wk, Wv)
    print("ka, kb:", ka, kb, "exp_shift:", exp_shift)
    nc = _build(ka, kb, exp_shift)
    print("built+compiled")

    sim = CoreSim(nc, trace=False)
    for name, arr in in_maps[0].items():
        sim.tensor(name)[:] = arr
    sim.simulate()
    got = np.array(sim.tensor("out"))

    q = queries @ Wq.T
    k = keys @ Wk.T
    worst = 0.0
    for slot, b in enumerate(assignment[0]):
        feats = np.tanh(q[b][:, None, :] + k[b][None, :, :])
        scores = feats @ Wv[0]
        vlb = int(valid_lens[b])
        scores[:, vlb:] = -1e6
        e = np.exp(scores - scores.max(-1, keepdims=True))
        attn = e / e.sum(-1, keepdims=True)
        exp_out = attn @ values[b]
        gj = got[slot]
        err = np.abs(gj - exp_out)
        rel = err.max() / np.abs(exp_out).max()
        worst = max(worst, rel)
        print(f"slot {slot} (b={b}, vl={vlb}): absmax-rel err {rel:.3e}")
    print("worst:", worst)


# revision 7
# speedup vs baseline: 1.1501x; 1.0391x over previous
"""Additive attention (B=16, Q=128, K=1024, D=256, H=64) on 8 trn2 NeuronCores.

scores[b,q,k] = sum_h Wv[h] * tanh(qproj[b,q,h] + kproj[b,k,h]); softmax over
valid k only; out = attn @ values.

v5 design (low-rank separable tanh): tanh(a+b) ~= sum_r phi_r(a) psi_r(b)
with phi/psi from a Gaussian-weighted SVD of tanh on a grid (R=10 ranks,
end-to-end output err ~3e-3).  This turns the whole O(B*Q*K*H) tanh+score
computation into one PE matmul with a 64*R=640 contraction:

  scores^T[k, q] = sum_m Bm[k, m] * A[q, m],   m = (r, h)
  A[q, m]  = Wv[h] * phi_r(qproj[q, h])        (host)
  Bm[k, m] = psi_r(kproj[k, h])                (host)

Ranks 0-3 (passes 0-1) ship in bf16; ranks 4-9 (passes 2-4) have tiny
amplitude and ship in fp8e4m3 (halves those bytes; output err unchanged at
~3e-3).  DMAs are few and large (128 descriptors each, partition-major
dram layouts) because each DMA trigger costs ~600ns on the issuing queue.

Device per batch (2 batches/core, data-parallel over B):
  - PE: per 128-k chunk, 5 matmuls (lhsT = Bm chunk [128m,128k], rhs =
    A [128m, 128q]) accumulate scores^T [128k, 128q] in psum
  - ACT: exp(psum) -> attn bf16 sbuf (k-major layout: no transpose needed)
  - PE: AV accumulation: lhsT = attn [128k, 128q], rhs = values_aug
    [128k, 258] -> [128q, 258] psum; col 256 is the ones column giving the
    softmax denominator; host zeroes rows >= valid_len so no masking needed
  - DVE: out = av[:, :256] * reciprocal(av[:, 256]); DMA store.

Batches are paired big+small by chunk count; compile shape (KA, KB) is the
max pair; shorter batches ship zero-padded chunks (scores 0 -> exp 1 ->
attn * zero values = 0, harmless).
"""

import sys

for _p in ("/opt/trn_rl_repo",):
    if _p not in sys.path:
        sys.path.append(_p)

import numpy as np
import ml_dtypes

import concourse.bass as bass  # noqa: F401
import concourse.tile as tile
from concourse import bacc, mybir
from concourse.bass_utils import run_bass_kernel_spmd

F32 = mybir.dt.float32
BF16 = mybir.dt.bfloat16
FP8 = mybir.dt.float8e4
BF = ml_dtypes.bfloat16
F8 = ml_dtypes.float8_e4m3

B, Q, K, D, H, V = 16, 128, 1024, 256, 64, 256
VW = 258          # 256 values + ones column + pad
NCORES = 8
R = 10            # separable rank
HR = H * R        # contraction size
NPASS = HR // 128  # matmul passes per score chunk
NBF = 2           # passes 0..NBF-1 in bf16, rest fp8
NF8 = NPASS - NBF
HEAD = 2          # slot-A chunks in the early Bm piece

_cache = {}
_basis_cache = {}


def _basis(sig):
    """Gaussian-weighted SVD of tanh(a+b) on a grid -> phi, psi [n, R]."""
    key = round(float(sig), 2)
    if key in _basis_cache:
        return _basis_cache[key]
    L = max(6.5, 5.5 * sig)
    n = 1301
    a = np.linspace(-L, L, n)
    p = np.exp(-a * a / (2.0 * sig * sig))
    w = np.sqrt(np.maximum(p, 3e-3))
    G = np.tanh(a[:, None] + a[None, :])
    M = w[:, None] * G * w[None, :]
    U, S, Vt = np.linalg.svd(M)
    phi = (U[:, :R] * np.sqrt(S[:R])) / w[:, None]
    psi = (Vt[:R].T * np.sqrt(S[:R])) / w[:, None]
    _basis_cache[key] = (L, n, a, phi, psi)
    return _basis_cache[key]


def _interp(x, L, n, F):
    idx = (np.clip(x, -L, L) + L) / (2 * L) * (n - 1)
    i0 = np.clip(idx.astype(np.int64), 0, n - 2)
    fr = (idx - i0)[..., None]
    return F[i0] * (1.0 - fr) + F[i0 + 1] * fr


def _build(ka, kb, exp_shift):
    nc = bacc.Bacc("TRN2", target_bir_lowering=False, debug=False,
                   num_devices=NCORES)
    kcs = (ka, kb)

    Ab_d = [nc.dram_tensor(f"Ab{b}", [128, NBF * 128], BF16,
                           kind="ExternalInput") for b in range(2)]
    A8_d = [nc.dram_tensor(f"A8{b}", [128, NF8 * 128], FP8,
                           kind="ExternalInput") for b in range(2)]
    Bb_d = [nc.dram_tensor(f"Bb{b}", [128, kcs[b], NBF, 128], BF16,
                           kind="ExternalInput") for b in range(2)]
    B8_d = [nc.dram_tensor(f"B8{b}", [128, kcs[b], NF8, 128], FP8,
                           kind="ExternalInput") for b in range(2)]
    vA_d = [nc.dram_tensor(f"vA{b}", [128, kcs[b], VW], BF16,
                           kind="ExternalInput") for b in range(2)]
    out_d = nc.dram_tensor("out", [2, 128, V], F32, kind="ExternalOutput")

    with tile.TileContext(nc) as tc:
        with (
            tc.tile_pool(name="sb_A", bufs=4) as sb_A,
            tc.tile_pool(name="sb_B", bufs=10) as sb_B,
            tc.tile_pool(name="sb_v", bufs=2) as sb_v,
            tc.tile_pool(name="sb_attn", bufs=4) as sb_attn,
            tc.tile_pool(name="sb_out", bufs=2) as sb_out,
            tc.tile_pool(name="ps_sc", bufs=4, space="PSUM") as ps_sc,
            tc.tile_pool(name="ps_av", bufs=2, space="PSUM") as ps_av,
        ):
            # Few, large DMAs (each ~600ns of trigger time on its queue).
            # scalar queue: A + values; sync queue: the Bm stream.  Slot A's
            # Bm is split head/tail so PE can start early.
            ab, a8, vt = [], [], []
            for b in range(2):
                t = sb_A.tile([128, NBF, 128], BF16, tag="ab", name=f"ab{b}")
                nc.scalar.dma_start(out=t, in_=Ab_d[b].ap().rearrange(
                    "p (s q) -> p s q", s=NBF))
                ab.append(t)
                t = sb_A.tile([128, NF8, 128], FP8, tag="a8", name=f"a8{b}")
                nc.scalar.dma_start(out=t, in_=A8_d[b].ap().rearrange(
                    "p (s q) -> p s q", s=NF8))
                a8.append(t)
                t = sb_v.tile([128, kcs[b], VW], BF16, tag="v", name=f"v{b}")
                nc.scalar.dma_start(out=t, in_=vA_d[b].ap())
                vt.append(t)
            bb = {}
            b8 = {}
            pieces = [(0, (lo, min(lo + 3, ka))) for lo in range(0, ka, 3)]
            pieces += [(1, (lo, min(lo + 3, kb))) for lo in range(0, kb, 3)]
            for b, (lo, hi) in pieces:
                t = sb_B.tile([128, hi - lo, NBF, 128], BF16, tag="bb",
                              name=f"bb{b}_{lo}")
                nc.sync.dma_start(out=t, in_=Bb_d[b].ap()[:, lo:hi])
                for c in range(lo, hi):
                    bb[(b, c)] = t[:, c - lo]
                t = sb_B.tile([128, hi - lo, NF8, 128], FP8, tag="b8",
                              name=f"b8{b}_{lo}")
                nc.sync.dma_start(out=t, in_=B8_d[b].ap()[:, lo:hi])
                for c in range(lo, hi):
                    b8[(b, c)] = t[:, c - lo]

            # software-pipelined: scores+exp of job i run while AV of job
            # i-DELAY consumes an older chunk's attn, so the PE never waits
            # on the ACT exp latency.
            DELAY = 2
            jobs = [(b, c) for b in range(2) for c in range(kcs[b])]
            avs = {b: ps_av.tile([128, VW], F32, tag="av", name=f"av{b}")
                   for b in range(2)}
            attns = {}

            def do_av(b, c):
                nc.tensor.matmul(
                    avs[b], attns.pop((b, c)), vt[b][:, c, :],
                    start=(c == 0), stop=(c == kcs[b] - 1))
                if c == kcs[b] - 1:
                    rcp = sb_out.tile([128, 1], F32, tag="rcp",
                                      name=f"rcp{b}")
                    nc.vector.reciprocal(rcp, avs[b][:, V:V + 1])
                    outt = sb_out.tile([128, V], F32, tag="out",
                                       name=f"out{b}")
                    nc.vector.tensor_scalar_mul(outt, avs[b][:, 0:V], rcp)
                    nc.sync.dma_start(out=out_d.ap()[b], in_=outt)

            for i, (b, c) in enumerate(jobs):
                sc = ps_sc.tile([128, 128], F32, tag="sc", name=f"sc{b}_{c}")
                for p in range(NBF):
                    nc.tensor.matmul(
                        sc, bb[(b, c)][:, p, :], ab[b][:, p, :],
                        start=(p == 0), stop=False)
                for p in range(NF8):
                    nc.tensor.matmul(
                        sc, b8[(b, c)][:, p, :], a8[b][:, p, :],
                        start=False, stop=(p == NF8 - 1))
                attn = sb_attn.tile([128, 128], BF16, tag="attn",
                                    name=f"attn{b}_{c}")
                nc.scalar.activation(
                    attn, sc, mybir.ActivationFunctionType.Exp,
                    bias=-exp_shift)
                attns[(b, c)] = attn
                if i >= DELAY:
                    do_av(*jobs[i - DELAY])
            for j in jobs[-DELAY:]:
                do_av(*j)

    nc.compile()
    return nc


def _prep(queries, keys, values, valid_lens, Wq, Wk, Wv):
    vl = [int(x) for x in np.asarray(valid_lens).reshape(-1)]
    assert len(vl) == B

    qf = np.asarray(queries, np.float32)
    kf = np.asarray(keys, np.float32)
    qproj = np.einsum('bqd,hd->bqh', qf, np.asarray(Wq, np.float32))
    kproj = np.einsum('bkd,hd->bkh', kf, np.asarray(Wk, np.float32))
    wv = np.asarray(Wv, np.float32).reshape(-1)                  # [H]

    sig = float(np.sqrt((qproj.var() + kproj.var()) / 2.0))
    sig = min(max(sig, 0.3), 4.0)
    L, n, _, phi, psi = _basis(sig)

    # A[b, m, q], Bm[b, m, k]; m = r*64 + h, A weighted by Wv
    phq = _interp(qproj, L, n, phi)          # [B, Q, H, R]
    Am = np.transpose(phq * wv[None, None, :, None],
                      (0, 3, 2, 1)).reshape(B, HR, Q)
    psk = _interp(kproj, L, n, psi)          # [B, K, H, R]
    Bmm = np.transpose(psk, (0, 3, 2, 1)).reshape(B, HR, K)

    bound = float(np.abs(wv).sum())
    exp_shift = max(0.0, bound - 30.0)

    # pair batches big+small by chunk count to minimize (KA, KB)
    kc_b = [(x + 127) // 128 for x in vl]
    order = sorted(range(B), key=lambda b: -kc_b[b])
    pairs = [(order[i], order[B - 1 - i]) for i in range(NCORES)]
    ka = max(kc_b[p[0]] for p in pairs)
    kb = max(kc_b[p[1]] for p in pairs)

    va = np.zeros((B, K, VW), BF)
    va[:, :, :V] = np.asarray(values, BF)
    va[:, :, V] = BF(1.0)
    for b in range(B):
        va[b, vl[b]:, :] = 0

    in_maps = []
    assignment = []
    for c in range(NCORES):
        m = {}
        for slot, (bbn, cap) in enumerate(zip(pairs[c], (ka, kb))):
            kc = kc_b[bbn]
            # A: [128i, s, 128q], element (i, s, q) = Am[b, s*128+i, q]
            a_arr = Am[bbn].reshape(NPASS, 128, Q).transpose(1, 0, 2)
            m[f"Ab{slot}"] = np.ascontiguousarray(
                a_arr[:, :NBF].reshape(128, NBF * Q)).astype(BF)
            m[f"A8{slot}"] = np.ascontiguousarray(
                a_arr[:, NBF:].reshape(128, NF8 * Q)).astype(F8)
            # Bm: [128i, cap, s, 128k'], (i, c, s, k') =
            #     Bmm[b, s*128+i, c*128+k']; zero-pad chunks >= kc
            src = Bmm[bbn].reshape(NPASS, 128, 8, 128)  # [s, i, c, k']
            b_arr = np.zeros((128, cap, NPASS, 128), np.float32)
            b_arr[:, :kc] = src.transpose(1, 2, 0, 3)[:, :kc]
            m[f"Bb{slot}"] = np.ascontiguousarray(b_arr[:, :, :NBF]).astype(BF)
            m[f"B8{slot}"] = np.ascontiguousarray(b_arr[:, :, NBF:]).astype(F8)
            # values: [128k', cap, VW], zero-padded
            v_arr = np.zeros((128, cap, VW), BF)
            v_arr[:, :kc] = va[bbn].reshape(8, 128, VW)[:kc].transpose(1, 0, 2)
            m[f"vA{slot}"] = v_arr
        in_maps.append(m)
        assignment.append(pairs[c])
    return ka, kb, exp_shift, in_maps, assignment


def kernel(queries, keys, values, valid_lens, Wq, Wk, Wv):
    ka, kb, exp_shift, in_maps, assignment = _prep(
        queries, keys, values, valid_lens, Wq, Wk, Wv)
    key = (ka, kb, round(exp_shift, 3))
    if key not in _cache:
        _cache[key] = _build(ka, kb, exp_shift)
    nc = _cache[key]
    res = run_bass_kernel_spmd(nc, in_maps, list(range(NCORES)))
    out = np.zeros((B, Q, V), np.float32)
    for c in range(NCORES):
        o = res.results[c]["out"]           # [2, 128, V]
        for slot, bbn in enumerate(assignment[c]):
            out[bbn] = o[slot]
    return out


if __name__ == "__main__":
    from concourse.bass_interp import CoreSim

    rng = np.random.default_rng(0)
    queries = rng.standard_normal((B, Q, D), np.float32)
    keys = rng.standard_normal((B, K, D), np.float32)
    values = rng.standard_normal((B, K, V), np.float32)
    valid_lens = rng.integers(1, K + 1, (B,)).astype(np.int64)
    Wq = (rng.standard_normal((H, D), np.float32) / np.sqrt(D)).astype(np.float32)
    Wk = (rng.standard_normal((H, D), np.float32) / np.sqrt(D)).astype(np.float32)
    Wv = (rng.standard_normal((1, H), np.float32) / np.sqrt(H)).astype(np.float32)

    ka, kb, exp_shift, in_maps, assignment = _prep(
        queries, keys, values, valid_lens, Wq, Wk, Wv)
    print("ka, kb:", ka, kb, "exp_shift:", exp_shift)
    nc = _build(ka, kb, exp_shift)
    print("built+compiled")

    sim = CoreSim(nc, trace=False)
    for name, arr in in_maps[0].items():
        sim.tensor(name)[:] = arr
    sim.simulate()
    got = np.array(sim.tensor("out"))

    q = queries @ Wq.T
    k = keys @ Wk.T
    worst = 0.0
    for slot, b in enumerate(assignment[0]):
        feats = np.tanh(q[b][:, None, :] + k[b][None, :, :])
        scores = feats @ Wv[0]
        vlb = int(valid_lens[b])
        scores[:, vlb:] = -1e6
        e = np.exp(scores - scores.max(-1, keepdims=True))
        attn = e / e.sum(-1, keepdims=True)
        exp_out = attn @ values[b]
        gj = got[slot]
        err = np.abs(gj - exp_out)
        rel = err.max() / np.abs(exp_out).max()
        worst = max(worst, rel)
        print(f"slot {slot} (b={b}, vl={vlb}): absmax-rel err {rel:.3e}")
    print("worst:", worst)


# revision 12
# speedup vs baseline: 1.2540x; 1.0903x over previous
"""Additive attention (B=16, Q=128, K=1024, D=256, H=64) on 8 trn2 NeuronCores.

scores[b,q,k] = sum_h Wv[h] * tanh(qproj[b,q,h] + kproj[b,k,h]); softmax over
valid k only; out = attn @ values.

v5 design (low-rank separable tanh): tanh(a+b) ~= sum_r phi_r(a) psi_r(b)
with phi/psi from a Gaussian-weighted SVD of tanh on a grid (R=10 ranks,
end-to-end output err ~3e-3).  This turns the whole O(B*Q*K*H) tanh+score
computation into one PE matmul with a 64*R=640 contraction:

  scores^T[k, q] = sum_m Bm[k, m] * A[q, m],   m = (r, h)
  A[q, m]  = Wv[h] * phi_r(qproj[q, h])        (host)
  Bm[k, m] = psi_r(kproj[k, h])                (host)

Ranks 0-3 (passes 0-1) ship in bf16; ranks 4-9 (passes 2-4) have tiny
amplitude and ship in fp8e4m3 (halves those bytes; output err unchanged at
~3e-3).  DMAs are few and large (128 descriptors each, partition-major
dram layouts) because each DMA trigger costs ~600ns on the issuing queue.

Device per batch (2 batches/core, data-parallel over B):
  - PE: per 128-k chunk, 5 matmuls (lhsT = Bm chunk [128m,128k], rhs =
    A [128m, 128q]) accumulate scores^T [128k, 128q] in psum
  - ACT: exp(psum) -> attn bf16 sbuf (k-major layout: no transpose needed)
  - PE: AV accumulation: lhsT = attn [128k, 128q], rhs = values_aug
    [128k, 258] -> [128q, 258] psum; col 256 is the ones column giving the
    softmax denominator; host zeroes rows >= valid_len so no masking needed
  - DVE: out = av[:, :256] * reciprocal(av[:, 256]); DMA store.

Batches are paired big+small by chunk count; compile shape (KA, KB) is the
max pair; shorter batches ship zero-padded chunks (scores 0 -> exp 1 ->
attn * zero values = 0, harmless).
"""

import sys

for _p in ("/opt/trn_rl_repo",):
    if _p not in sys.path:
        sys.path.append(_p)

import numpy as np
import ml_dtypes

import concourse.bass as bass  # noqa: F401
import concourse.tile as tile
from concourse import bacc, mybir
from concourse.bass_utils import run_bass_kernel_spmd

F32 = mybir.dt.float32
BF16 = mybir.dt.bfloat16
FP8 = mybir.dt.float8e4
BF = ml_dtypes.bfloat16
F8 = ml_dtypes.float8_e4m3

B, Q, K, D, H, V = 16, 128, 1024, 256, 64, 256
VW = 258          # 256 values + ones column + pad
NCORES = 8
R = 10            # separable rank
HR = H * R        # contraction size
NPASS = HR // 128  # matmul passes per score chunk
NBF = 2           # passes 0..NBF-1 in bf16, rest fp8
NF8 = NPASS - NBF
HEAD = 2          # slot-A chunks in the early Bm piece

_cache = {}
_basis_cache = {}


def _basis(sig):
    """Gaussian-weighted SVD of tanh(a+b) on a grid -> phi, psi [n, R]."""
    key = round(float(sig), 2)
    if key in _basis_cache:
        return _basis_cache[key]
    L = max(6.5, 5.5 * sig)
    n = 1301
    a = np.linspace(-L, L, n)
    p = np.exp(-a * a / (2.0 * sig * sig))
    w = np.sqrt(np.maximum(p, 3e-3))
    G = np.tanh(a[:, None] + a[None, :])
    M = w[:, None] * G * w[None, :]
    U, S, Vt = np.linalg.svd(M)
    phi = (U[:, :R] * np.sqrt(S[:R])) / w[:, None]
    psi = (Vt[:R].T * np.sqrt(S[:R])) / w[:, None]
    _basis_cache[key] = (L, n, a, phi, psi)
    return _basis_cache[key]


def _interp(x, L, n, F):
    idx = (np.clip(x, -L, L) + L) / (2 * L) * (n - 1)
    i0 = np.clip(idx.astype(np.int64), 0, n - 2)
    fr = (idx - i0)[..., None]
    return F[i0] * (1.0 - fr) + F[i0 + 1] * fr


def _build(ka, kb, exp_shift):
    nc = bacc.Bacc("TRN2", target_bir_lowering=False, debug=False,
                   num_devices=NCORES)
    kcs = (ka, kb)

    Ab_d = [nc.dram_tensor(f"Ab{b}", [128, NBF * 128], BF16,
                           kind="ExternalInput") for b in range(2)]
    A8_d = [nc.dram_tensor(f"A8{b}", [128, NF8 * 128], FP8,
                           kind="ExternalInput") for b in range(2)]
    Bb_d = [nc.dram_tensor(f"Bb{b}", [128, kcs[b], NBF, 128], BF16,
                           kind="ExternalInput") for b in range(2)]
    B8_d = [nc.dram_tensor(f"B8{b}", [128, kcs[b], NF8, 128], FP8,
                           kind="ExternalInput") for b in range(2)]
    vA_d = [nc.dram_tensor(f"vA{b}", [128, kcs[b], VW], BF16,
                           kind="ExternalInput") for b in range(2)]
    out_d = nc.dram_tensor("out", [2, 128, V], F32, kind="ExternalOutput")

    # Raw bass (no TileContext): static allocation, hand-rolled semaphores.
    # Tile's exit path costs two serial all-engine barriers (~3us each);
    # this kernel has a simple static dependency graph, so explicit sems
    # shave ~10us of framework pre/postamble.
    sAx = [nc.alloc_semaphore(f"sA{b}") for b in range(2)]   # A loads
    sVx = [nc.alloc_semaphore(f"sV{b}") for b in range(2)]   # values loads
    sSC = nc.alloc_semaphore("sSC")  # score psum group done -> exp
    sEXP = nc.alloc_semaphore("sE")  # exp done -> AV / bank reuse
    sAV = nc.alloc_semaphore("sAV")  # AV group done -> norm
    sN = nc.alloc_semaphore("sN")    # norm done -> store
    sR = nc.alloc_semaphore("sR")    # reciprocal write visible -> mul
    sD = nc.alloc_semaphore("sD")    # stores done

    ab = [nc.alloc_sbuf_tensor(f"ab{b}", [128, NBF, 128], BF16).ap()
          for b in range(2)]
    a8 = [nc.alloc_sbuf_tensor(f"a8{b}", [128, NF8, 128], FP8).ap()
          for b in range(2)]
    vt = [nc.alloc_sbuf_tensor(f"v{b}", [128, kcs[b], VW], BF16).ap()
          for b in range(2)]
    jobs = [(b, c) for b in range(2) for c in range(kcs[b])]
    attn_t = [nc.alloc_sbuf_tensor(f"at{i}", [128, 128], BF16).ap()
              for i in range(len(jobs))]
    rcp_t = [nc.alloc_sbuf_tensor(f"rcp{b}", [128, 1], F32).ap()
             for b in range(2)]
    out_t = [nc.alloc_sbuf_tensor(f"ot{b}", [128, V], F32).ap()
             for b in range(2)]

    NSC = 6
    scb = [nc.place_psum_tensor(f"sc{i}", [128, 128], F32, bank=i).ap()
           for i in range(NSC)]
    avb = [nc.place_psum_tensor(f"av{b}", [128, VW], F32, bank=NSC + b).ap()
           for b in range(2)]

    if exp_shift:
        sC = nc.alloc_semaphore("sC")
        bias_sb = nc.alloc_sbuf_tensor("bias", [128, 1], F32).ap()
        nc.gpsimd.memset(bias_sb, -exp_shift).then_inc(sC, 1)
        bias_arg = bias_sb
    else:
        sC = None
        bias_arg = 0.0

    # Bm pieces of up to 3 chunks; map job -> piece ordinal
    pieces = [(0, lo, min(lo + 3, ka)) for lo in range(0, ka, 3)]
    pieces += [(1, lo, min(lo + 3, kb)) for lo in range(0, kb, 3)]
    bb, b8t = {}, {}
    piece_of = {}
    piece_aps = []
    for pi, (b, lo, hi) in enumerate(pieces):
        tb = nc.alloc_sbuf_tensor(f"bb{b}_{lo}", [128, hi - lo, NBF, 128],
                                  BF16).ap()
        t8 = nc.alloc_sbuf_tensor(f"b8{b}_{lo}", [128, hi - lo, NF8, 128],
                                  FP8).ap()
        piece_aps.append((tb, t8))
        for c in range(lo, hi):
            bb[(b, c)] = tb[:, c - lo]
            b8t[(b, c)] = t8[:, c - lo]
            piece_of[(b, c)] = pi

    # ---- SYNC engine stream: Bm loads, then output stores ----
    sBp = [nc.alloc_semaphore(f"sB{pi}") for pi in range(len(pieces))]
    for pi, (b, lo, hi) in enumerate(pieces):
        tb, t8 = piece_aps[pi]
        nc.sync.dma_start(out=tb,
                          in_=Bb_d[b].ap()[:, lo:hi]).then_inc(sBp[pi], 16)
        nc.sync.dma_start(out=t8,
                          in_=B8_d[b].ap()[:, lo:hi]).then_inc(sBp[pi], 16)
    for b in range(2):
        nc.sync.wait_ge(sN, b + 1)
        nc.sync.dma_start(out=out_d.ap()[b], in_=out_t[b]).then_inc(sD, 16)
    nc.sync.wait_ge(sD, 32)
    nc.sync.drain()

    # ---- SCALAR engine stream: A + values loads, then exps ----
    for b in range(2):
        nc.scalar.dma_start(out=ab[b], in_=Ab_d[b].ap().rearrange(
            "p (s q) -> p s q", s=NBF)).then_inc(sAx[b], 16)
        nc.scalar.dma_start(out=a8[b], in_=A8_d[b].ap().rearrange(
            "p (s q) -> p s q", s=NF8)).then_inc(sAx[b], 16)
        nc.scalar.dma_start(out=vt[b], in_=vA_d[b].ap()).then_inc(sVx[b], 16)
    if sC is not None:
        nc.scalar.wait_ge(sC, 1)
    for i, (b, c) in enumerate(jobs):
        nc.scalar.wait_ge(sSC, i + 1)
        nc.scalar.activation(
            attn_t[i], scb[i % NSC], mybir.ActivationFunctionType.Exp,
            bias=bias_arg).then_inc(sEXP, 1)

    # ---- TENSOR engine stream: scores + AV, software-pipelined ----
    DELAY = 2
    seen_piece = -1
    seen_av = {0: False, 1: False}

    def do_av(i):
        b, c = jobs[i]
        if not seen_av[b]:
            nc.tensor.wait_ge(sVx[b], 16)
            seen_av[b] = True
        nc.tensor.wait_ge(sEXP, i + 1)
        mm = nc.tensor.matmul(avb[b], attn_t[i], vt[b][:, c, :],
                              start=(c == 0), stop=(c == kcs[b] - 1))
        if c == kcs[b] - 1:
            mm.then_inc(sAV, 1)

    nc.tensor.wait_ge(sAx[0], 32)
    for i, (b, c) in enumerate(jobs):
        if b == 1 and c == 0:
            nc.tensor.wait_ge(sAx[1], 32)
        pi = piece_of[(b, c)]
        if pi > seen_piece:
            nc.tensor.wait_ge(sBp[pi], 32)
            seen_piece = pi
        if i >= NSC:
            nc.tensor.wait_ge(sEXP, i - NSC + 1)
        sc = scb[i % NSC]
        for p in range(NBF):
            nc.tensor.matmul(sc, bb[(b, c)][:, p, :], ab[b][:, p, :],
                             start=(p == 0), stop=False)
        for p in range(NF8):
            mm = nc.tensor.matmul(sc, b8t[(b, c)][:, p, :], a8[b][:, p, :],
                                  start=False, stop=(p == NF8 - 1))
            if p == NF8 - 1:
                mm.then_inc(sSC, 1)
        if i >= DELAY:
            do_av(i - DELAY)
    for i in range(len(jobs) - DELAY, len(jobs)):
        do_av(i)

    # ---- VECTOR engine stream: normalize ----
    for b in range(2):
        nc.vector.wait_ge(sAV, b + 1)
        nc.vector.reciprocal(rcp_t[b], avb[b][:, V:V + 1]).then_inc(sR, 1)
        nc.vector.wait_ge(sR, b + 1)
        nc.vector.tensor_scalar_mul(out_t[b], avb[b][:, 0:V],
                                    rcp_t[b]).then_inc(sN, 1)

    nc.compile()
    return nc


def _prep(queries, keys, values, valid_lens, Wq, Wk, Wv):
    vl = [int(x) for x in np.asarray(valid_lens).reshape(-1)]
    assert len(vl) == B

    qf = np.asarray(queries, np.float32)
    kf = np.asarray(keys, np.float32)
    qproj = np.einsum('bqd,hd->bqh', qf, np.asarray(Wq, np.float32))
    kproj = np.einsum('bkd,hd->bkh', kf, np.asarray(Wk, np.float32))
    wv = np.asarray(Wv, np.float32).reshape(-1)                  # [H]

    sig = float(np.sqrt((qproj.var() + kproj.var()) / 2.0))
    sig = min(max(sig, 0.3), 4.0)
    L, n, _, phi, psi = _basis(sig)

    # A[b, m, q], Bm[b, m, k]; m = r*64 + h, A weighted by Wv
    phq = _interp(qproj, L, n, phi)          # [B, Q, H, R]
    Am = np.transpose(phq * wv[None, None, :, None],
                      (0, 3, 2, 1)).reshape(B, HR, Q)
    psk = _interp(kproj, L, n, psi)          # [B, K, H, R]
    Bmm = np.transpose(psk, (0, 3, 2, 1)).reshape(B, HR, K)

    bound = float(np.abs(wv).sum())
    exp_shift = max(0.0, bound - 30.0)

    # pair batches big+small by chunk count to minimize (KA, KB)
    kc_b = [(x + 127) // 128 for x in vl]
    order = sorted(range(B), key=lambda b: -kc_b[b])
    pairs = [(order[i], order[B - 1 - i]) for i in range(NCORES)]
    ka = max(kc_b[p[0]] for p in pairs)
    kb = max(kc_b[p[1]] for p in pairs)

    va = np.zeros((B, K, VW), BF)
    va[:, :, :V] = np.asarray(values, BF)
    va[:, :, V] = BF(1.0)
    for b in range(B):
        va[b, vl[b]:, :] = 0

    in_maps = []
    assignment = []
    for c in range(NCORES):
        m = {}
        for slot, (bbn, cap) in enumerate(zip(pairs[c], (ka, kb))):
            kc = kc_b[bbn]
            # A: [128i, s, 128q], element (i, s, q) = Am[b, s*128+i, q]
            a_arr = Am[bbn].reshape(NPASS, 128, Q).transpose(1, 0, 2)
            m[f"Ab{slot}"] = np.ascontiguousarray(
                a_arr[:, :NBF].reshape(128, NBF * Q)).astype(BF)
            m[f"A8{slot}"] = np.ascontiguousarray(
                a_arr[:, NBF:].reshape(128, NF8 * Q)).astype(F8)
            # Bm: [128i, cap, s, 128k'], (i, c, s, k') =
            #     Bmm[b, s*128+i, c*128+k']; zero-pad chunks >= kc
            src = Bmm[bbn].reshape(NPASS, 128, 8, 128)  # [s, i, c, k']
            b_arr = np.zeros((128, cap, NPASS, 128), np.float32)
            b_arr[:, :kc] = src.transpose(1, 2, 0, 3)[:, :kc]
            m[f"Bb{slot}"] = np.ascontiguousarray(b_arr[:, :, :NBF]).astype(BF)
            m[f"B8{slot}"] = np.ascontiguousarray(b_arr[:, :, NBF:]).astype(F8)
            # values: [128k', cap, VW], zero-padded
            v_arr = np.zeros((128, cap, VW), BF)
            v_arr[:, :kc] = va[bbn].reshape(8, 128, VW)[:kc].transpose(1, 0, 2)
            m[f"vA{slot}"] = v_arr
        in_maps.append(m)
        assignment.append(pairs[c])
    return ka, kb, exp_shift, in_maps, assignment


def kernel(queries, keys, values, valid_lens, Wq, Wk, Wv):
    ka, kb, exp_shift, in_maps, assignment = _prep(
        queries, keys, values, valid_lens, Wq, Wk, Wv)
    key = (ka, kb, round(exp_shift, 3))
    if key not in _cache:
        _cache[key] = _build(ka, kb, exp_shift)
    nc = _cache[key]
    res = run_bass_kernel_spmd(nc, in_maps, list(range(NCORES)))
    out = np.zeros((B, Q, V), np.float32)
    for c in range(NCORES):
        o = res.results[c]["out"]           # [2, 128, V]
        for slot, bbn in enumerate(assignment[c]):
            out[bbn] = o[slot]
    return out


if __name__ == "__main__":
    from concourse.bass_interp import CoreSim

    rng = np.random.default_rng(0)
    queries = rng.standard_normal((B, Q, D), np.float32)
    keys = rng.standard_normal((B, K, D), np.float32)
    values = rng.standard_normal((B, K, V), np.float32)
    valid_lens = rng.integers(1, K + 1, (B,)).astype(np.int64)
    Wq = (rng.standard_normal((H, D), np.float32) / np.sqrt(D)).astype(np.float32)
    Wk = (rng.standard_normal((H, D), np.float32) / np.sqrt(D)).astype(np.float32)
    Wv = (rng.standard_normal((1, H), np.float32) / np.sqrt(H)).astype(np.float32)

    ka, kb, exp_shift, in_maps, assignment = _prep(
        queries, keys, values, valid_lens, Wq, Wk, Wv)
    print("ka, kb:", ka, kb, "exp_shift:", exp_shift)
    nc = _build(ka, kb, exp_shift)
    print("built+compiled")

    sim = CoreSim(nc, trace=False)
    for name, arr in in_maps[0].items():
        sim.tensor(name)[:] = arr
    sim.simulate()
    got = np.array(sim.tensor("out"))

    q = queries @ Wq.T
    k = keys @ Wk.T
    worst = 0.0
    for slot, b in enumerate(assignment[0]):
        feats = np.tanh(q[b][:, None, :] + k[b][None, :, :])
        scores = feats @ Wv[0]
        vlb = int(valid_lens[b])
        scores[:, vlb:] = -1e6
        e = np.exp(scores - scores.max(-1, keepdims=True))
        attn = e / e.sum(-1, keepdims=True)
        exp_out = attn @ values[b]
        gj = got[slot]
        err = np.abs(gj - exp_out)
        rel = err.max() / np.abs(exp_out).max()
        worst = max(worst, rel)
        print(f"slot {slot} (b={b}, vl={vlb}): absmax-rel err {rel:.3e}")
    print("worst:", worst)
